# revision 7
# baseline (speedup 1.0000x reference)
"""Explore-Recommendation-Decoder kernel for 8 Trainium2 NeuronCores.

Strategy (vocab-parallel + data-parallel attention):
  - Each core owns a 12500-wide vocab shard of E_w (kept resident in SBUF as
    bf16) and computes attention for its own 128-row batch chunk.
  - ctx^T (the attention-pooled memory, transposed to [H, B]) is AllGathered
    (bf16, 64KB) so every core has the full cat^T = [ctx^T; last_memory^T]
    needed as matmul lhsT for all batch chunks.
  - Per batch chunk r: logits = cat^T.T @ E_wT via 25 accumulating matmul
    pairs (K=256 split in two 128-K tiles), exp on ACT with fused per-row
    accumulation, a tiny AllGather of per-core corrected row sums, reciprocal
    and scale on DVE, f32 DMA to a per-chunk output tensor.
  - Masking of seen items (seq_item>0): the masked items' E columns are
    appended as extra matmul columns; exp of those logits dotted with a host
    0/1 selection matrix (tensor_tensor_reduce) gives the exact amount to
    subtract from each row's local sum. The masked output positions are
    zeroed with an indirect-DMA scatter into the chunk's output tensor.

Softmax over the 100k vocab needs no max subtraction: logits are |x| <~ 1 by
construction (cat entries bounded, E_w ~ N(0, 0.02)), far from f32 overflow.
"""
import numpy as np

import concourse.bacc as bacc
import concourse.bass as bass
import concourse.mybir as mybir
import concourse.tile as tile
from concourse.bass_utils import run_bass_kernel_spmd
from concourse.masks import make_identity

N_CORES = 8
B, S, H, V = 1024, 50, 128, 100000
VS = V // N_CORES            # 12500 vocab columns per core
BC = B // N_CORES            # 128 batch rows per chunk (= per core for attention)
NCH = N_CORES                # batch chunks
NW = 500                     # logits matmul free-dim tile (one PSUM bank)
NJ = VS // NW                # 25 n-chunks
BS = BC * S                  # 6400 flattened (b, s) pairs per chunk
AMW = 500                    # attention matmul tile: 10 batch rows x 50
F32 = mybir.dt.float32
BF16 = mybir.dt.bfloat16
I32 = mybir.dt.int32
AF = mybir.ActivationFunctionType
OP = mybir.AluOpType

_cache = {}


def _build(w_ext: int, n_scat: int):
    """Build the SPMD program. w_ext: extra-column block width per chunk
    (multiple of 448). n_scat: zero-scatter calls per chunk (128 idx each)."""
    nc = bacc.Bacc(None, target_bir_lowering=False, num_devices=N_CORES)

    # ---- inputs (per-core shards, prepared host-side) ----
    amT = nc.dram_tensor("amT", [H, BS], F32, kind="ExternalInput")
    am_bs = nc.dram_tensor("am_bs", [BC, S * H], F32, kind="ExternalInput")
    lastT_own = nc.dram_tensor("lastT_own", [H, BC], F32, kind="ExternalInput")
    lastT_bf = nc.dram_tensor("lastT_bf", [H, B], BF16, kind="ExternalInput")
    UeT = nc.dram_tensor("UeT", [H, H], F32, kind="ExternalInput")
    WeT = nc.dram_tensor("WeT", [H, H], F32, kind="ExternalInput")
    bias_col = nc.dram_tensor("bias_col", [H, 1], F32, kind="ExternalInput")
    VeCol = nc.dram_tensor("VeCol", [H, 1], F32, kind="ExternalInput")
    E0 = nc.dram_tensor("E0", [H, VS], BF16, kind="ExternalInput")
    E1 = nc.dram_tensor("E1", [H, VS], BF16, kind="ExternalInput")
    Eex0 = nc.dram_tensor("Eex0", [H, NCH * w_ext], BF16, kind="ExternalInput")
    Eex1 = nc.dram_tensor("Eex1", [H, NCH * w_ext], BF16, kind="ExternalInput")
    Dmask = nc.dram_tensor("Dmask", [BC, NCH * w_ext], BF16, kind="ExternalInput")
    scat = nc.dram_tensor("scat", [BC, NCH * n_scat], I32, kind="ExternalInput")

    outs = [
        nc.dram_tensor(f"out{r}", [BC, VS], F32, kind="ExternalOutput")
        for r in range(NCH)
    ]

    with tile.TileContext(nc) as tc:
        with (
            tc.tile_pool(name="const", bufs=1) as cp,
            tc.tile_pool(name="dram", bufs=1, space="DRAM") as dp,
        ):
            # resident weights / small constants
            e0_t = cp.tile([H, VS], BF16)
            nc.sync.dma_start(out=e0_t[:], in_=E0[:])
            e1_t = cp.tile([H, VS], BF16)
            nc.sync.dma_start(out=e1_t[:], in_=E1[:])
            lastT_t = cp.tile([H, B], BF16)
            nc.sync.dma_start(out=lastT_t[:], in_=lastT_bf[:])
            ident = cp.tile([H, H], F32)
            make_identity(nc, ident[:])
            zeros = cp.tile([BC, 1], F32)
            nc.gpsimd.memset(zeros[:], 0.0)
            scat_t = cp.tile([BC, NCH * n_scat], I32)
            nc.gpsimd.dma_start(out=scat_t[:], in_=scat[:])
            ctxT_all = cp.tile([H, NCH * BC], BF16)  # filled after AllGather

            # collective buffers
            ag_send = dp.tile([H, BC], BF16)
            ag_recv = dp.tile([NCH * H, BC], BF16, addr_space="Shared")
            sends = [dp.tile([BC, 1], F32, name=f"send{r}") for r in range(NCH)]
            recvs = [
                dp.tile([N_CORES * BC, 1], F32, addr_space="Shared", name=f"recv{r}")
                for r in range(NCH)
            ]

            # ---------------- attention for own batch chunk ----------------
            with (
                tc.tile_pool(name="attn", bufs=1) as ap,
                tc.tile_pool(name="attn_ps", bufs=3, space="PSUM") as aps,
                tc.tile_pool(name="attn_ps1", bufs=2, space="PSUM") as aps1,
            ):
                ueT_t = ap.tile([H, H], F32)
                nc.sync.dma_start(out=ueT_t[:], in_=UeT[:])
                weT_t = ap.tile([H, H], F32)
                nc.sync.dma_start(out=weT_t[:], in_=WeT[:])
                bias_t = ap.tile([H, 1], F32)
                nc.sync.dma_start(out=bias_t[:], in_=bias_col[:])
                ve_t = ap.tile([H, 1], F32)
                nc.sync.dma_start(out=ve_t[:], in_=VeCol[:])
                lown_t = ap.tile([H, BC], F32)
                nc.sync.dma_start(out=lown_t[:], in_=lastT_own[:])
                amT_t = ap.tile([H, BS], F32)
                nc.sync.dma_start(out=amT_t[:], in_=amT[:])
                ambs_t = ap.tile([BC, S * H], F32)
                nc.sync.dma_start(out=ambs_t[:], in_=am_bs[:])

                # lm_T[k, b] = We @ last + (We_b + Ue_b)
                lm_ps = aps1.tile([H, BC], F32, space="PSUM", tag="lmps")
                nc.tensor.matmul(
                    out=lm_ps[:], lhsT=weT_t[:], rhs=lown_t[:], start=True, stop=True
                )
                lm_sb = ap.tile([H, BC], F32)
                nc.vector.tensor_scalar_add(
                    out=lm_sb[:], in0=lm_ps[:], scalar1=bias_t[:, :1]
                )

                # h_T = tanh(Ue @ am + lm), scores = Ve . h_T
                h_sb = ap.tile([H, BS], F32)
                scores_row = ap.tile([1, BS], F32)
                scores_bs = ap.tile([BC, S], F32)
                for j in range((BS + AMW - 1) // AMW):
                    w = min(AMW, BS - j * AMW)
                    nb = w // S
                    b0 = j * AMW // S
                    sl = slice(j * AMW, j * AMW + w)
                    am_ps = aps.tile([H, AMW], F32, space="PSUM", tag="amps")
                    nc.tensor.matmul(
                        out=am_ps[:, :w], lhsT=ueT_t[:], rhs=amT_t[:, sl],
                        start=True, stop=True,
                    )
                    nc.vector.tensor_tensor(
                        out=h_sb[:, sl].rearrange("p (b s) -> p b s", s=S),
                        in0=am_ps[:, :w].rearrange("p (b s) -> p b s", s=S),
                        in1=lm_sb[:, b0:b0 + nb, None].to_broadcast([H, nb, S]),
                        op=OP.add,
                    )
                    nc.scalar.activation(
                        out=h_sb[:, sl], in_=h_sb[:, sl], func=AF.Tanh
                    )
                    sc_ps = aps1.tile([1, AMW], F32, space="PSUM", tag="scps")
                    nc.tensor.matmul(
                        out=sc_ps[:1, :w], lhsT=ve_t[:], rhs=h_sb[:, sl],
                        start=True, stop=True,
                    )
                    nc.scalar.copy(out=scores_row[:1, sl], in_=sc_ps[:1, :w])
                    nc.sync.dma_start(
                        out=scores_bs[b0:b0 + nb, :],
                        in_=scores_row[0:1, sl].rearrange(
                            "p (b s) -> p b s", s=S),
                    )

                # softmax over s per row
                negmax = ap.tile([BC, 1], F32)
                nc.vector.reduce_max(
                    out=negmax[:], in_=scores_bs[:], axis=mybir.AxisListType.X,
                    negate=True,
                )
                attn = ap.tile([BC, S], F32)
                ssum = ap.tile([BC, 1], F32)
                nc.scalar.activation(
                    out=attn[:], in_=scores_bs[:], func=AF.Exp,
                    bias=negmax[:, :1], accum_out=ssum[:],
                )
                rsum = ap.tile([BC, 1], F32)
                nc.vector.reciprocal(out=rsum[:], in_=ssum[:])
                nc.vector.tensor_scalar_mul(
                    out=attn[:], in0=attn[:], scalar1=rsum[:, :1]
                )

                # ctx[b, h] = sum_s attn[b, s] * all_memory[b, s, h]
                nc.vector.tensor_tensor(
                    out=ambs_t[:].rearrange("p (s h) -> p s h", h=H),
                    in0=ambs_t[:].rearrange("p (s h) -> p s h", h=H),
                    in1=attn[:, :, None].to_broadcast([BC, S, H]),
                    op=OP.mult,
                )
                ctx = ap.tile([BC, H], F32)
                nc.vector.reduce_sum(
                    out=ctx[:], in_=ambs_t[:].rearrange("p (s h) -> p h s", h=H),
                    axis=mybir.AxisListType.X,
                )

                # ctx^T -> bf16 -> AllGather
                ctxT_ps = aps1.tile([H, BC], F32, space="PSUM", tag="lmps")
                nc.tensor.transpose(out=ctxT_ps[:], in_=ctx[:], identity=ident[:])
                ctxT_bf = ap.tile([H, BC], BF16)
                nc.vector.tensor_copy(out=ctxT_bf[:], in_=ctxT_ps[:])
                nc.sync.dma_start(out=ag_send[:], in_=ctxT_bf[:])
                nc.gpsimd.collective_compute(
                    "AllGather", OP.bypass,
                    replica_groups=[list(range(N_CORES))],
                    ins=[ag_send[:]], outs=[ag_recv[:]],
                )
                nc.sync.dma_start(
                    out=ctxT_all[:].rearrange("p (r b) -> p r b", r=NCH),
                    in_=ag_recv[:].rearrange("(r h) b -> h r b", h=H),
                )

            # ---------------- vocab phase, one batch chunk at a time --------
            with (
                tc.tile_pool(name="exp", bufs=2) as ep,
                tc.tile_pool(name="ex", bufs=2) as xp,
                tc.tile_pool(name="mm_ps", bufs=6, space="PSUM") as mps,
                tc.tile_pool(name="ex_ps", bufs=2, space="PSUM") as xps,
            ):
                n_ex = w_ext // 448
                for r in range(NCH):
                    k0 = ctxT_all[:, r * BC:(r + 1) * BC]
                    k1 = lastT_t[:, r * BC:(r + 1) * BC]
                    exp_sb = ep.tile([BC, VS], F32, tag="exp")
                    sums = xp.tile([BC, NJ], F32, tag="sums")
                    for j in range(NJ):
                        sl = slice(j * NW, (j + 1) * NW)
                        ps = mps.tile([BC, NW], F32, space="PSUM", tag="mm")
                        nc.tensor.matmul(
                            out=ps[:], lhsT=k0, rhs=e0_t[:, sl],
                            start=True, stop=False,
                        )
                        nc.tensor.matmul(
                            out=ps[:], lhsT=k1, rhs=e1_t[:, sl],
                            start=False, stop=True,
                        )
                        nc.scalar.activation(
                            out=exp_sb[:, sl], in_=ps[:], func=AF.Exp,
                            accum_out=sums[:, j:j + 1],
                        )

                    # extra (masked-item) columns: recompute their exp to
                    # subtract from the local row sums
                    eex0_t = xp.tile([H, w_ext], BF16, tag="eex0")
                    nc.sync.dma_start(
                        out=eex0_t[:], in_=Eex0[:, r * w_ext:(r + 1) * w_ext]
                    )
                    eex1_t = xp.tile([H, w_ext], BF16, tag="eex1")
                    nc.sync.dma_start(
                        out=eex1_t[:], in_=Eex1[:, r * w_ext:(r + 1) * w_ext]
                    )
                    dmask_b = xp.tile([BC, w_ext], BF16, tag="dmaskb")
                    nc.sync.dma_start(
                        out=dmask_b[:], in_=Dmask[:, r * w_ext:(r + 1) * w_ext]
                    )
                    dmask_t = xp.tile([BC, w_ext], F32, tag="dmask")
                    nc.vector.tensor_copy(out=dmask_t[:], in_=dmask_b[:])
                    exp_ex = xp.tile([BC, w_ext], F32, tag="expex")
                    for x in range(n_ex):
                        xsl = slice(x * 448, (x + 1) * 448)
                        psx = xps.tile([BC, 448], F32, space="PSUM", tag="ex")
                        nc.tensor.matmul(
                            out=psx[:], lhsT=k0, rhs=eex0_t[:, xsl],
                            start=True, stop=False,
                        )
                        nc.tensor.matmul(
                            out=psx[:], lhsT=k1, rhs=eex1_t[:, xsl],
                            start=False, stop=True,
                        )
                        nc.scalar.activation(
                            out=exp_ex[:, xsl], in_=psx[:], func=AF.Exp
                        )
                    # NOTE: tensor_tensor_reduce hangs the HW here; use
                    # separate mult + reduce instead.
                    ttr_o = xp.tile([BC, w_ext], F32, tag="ttro")
                    corr = xp.tile([BC, 1], F32, tag="corr")
                    nc.vector.tensor_tensor(
                        out=ttr_o[:], in0=exp_ex[:], in1=dmask_t[:], op=OP.mult
                    )
                    nc.vector.reduce_sum(
                        out=corr[:], in_=ttr_o[:], axis=mybir.AxisListType.X
                    )
                    ls = xp.tile([BC, 1], F32, tag="ls")
                    nc.vector.reduce_sum(
                        out=ls[:], in_=sums[:], axis=mybir.AxisListType.X
                    )
                    corrected = xp.tile([BC, 1], F32, tag="cd")
                    nc.vector.tensor_sub(
                        out=corrected[:], in0=ls[:], in1=corr[:]
                    )

                    # share per-core row sums, derive 1/global_sum
                    nc.sync.dma_start(out=sends[r][:], in_=corrected[:])
                    nc.gpsimd.collective_compute(
                        "AllGather", OP.bypass,
                        replica_groups=[list(range(N_CORES))],
                        ins=[sends[r][:]], outs=[recvs[r][:]],
                    )
                    rsums = xp.tile([BC, N_CORES], F32, tag="rsums")
                    nc.sync.dma_start(
                        out=rsums[:],
                        in_=recvs[r][:, 0].rearrange("(r b) -> b r", b=BC),
                    )
                    gsum = xp.tile([BC, 1], F32, tag="gsum")
                    nc.vector.reduce_sum(
                        out=gsum[:], in_=rsums[:], axis=mybir.AxisListType.X
                    )
                    inv = xp.tile([BC, 1], F32, tag="inv")
                    nc.vector.reciprocal(out=inv[:], in_=gsum[:])

                    # scale + write, quarter at a time for overlap
                    q = VS // 4
                    for x in range(4):
                        qsl = slice(x * q, (x + 1) * q)
                        nc.vector.tensor_scalar_mul(
                            out=exp_sb[:, qsl], in0=exp_sb[:, qsl],
                            scalar1=inv[:, :1],
                        )
                        nc.sync.dma_start(out=outs[r][:, qsl], in_=exp_sb[:, qsl])

                    # zero the masked positions in the written chunk
                    for k in range(n_scat):
                        col = r * n_scat + k
                        nc.gpsimd.indirect_dma_start(
                            out=outs[r][:].rearrange(
                                "p (v one) -> (p v) one", one=1
                            ),
                            out_offset=bass.IndirectOffsetOnAxis(
                                ap=scat_t[:, col:col + 1], axis=0
                            ),
                            in_=zeros[:],
                            in_offset=None,
                            bounds_check=BC * VS - 1,
                            oob_is_err=False,
                        )
    nc.compile()
    return nc


def _prep_inputs(all_memory, last_memory, seq_item, Ue_w, Ue_b, We_w, We_b,
                 Ve_w, Ve_b, E_w):
    f32 = np.float32
    all_memory = np.asarray(all_memory, dtype=f32)
    last_memory = np.asarray(last_memory, dtype=f32)
    seq_item = np.asarray(seq_item)
    E_w = np.asarray(E_w, dtype=f32)

    # masked (b, v) pairs per (core, chunk), deduped
    items = seq_item.astype(np.int64)
    valid = items > 0
    masked = [[None] * NCH for _ in range(N_CORES)]
    max_m = 0
    for c in range(N_CORES):
        lo, hi = c * VS, (c + 1) * VS
        for r in range(NCH):
            it = items[r * BC:(r + 1) * BC]
            va = valid[r * BC:(r + 1) * BC]
            b_idx, s_idx = np.nonzero(va & (it >= lo) & (it < hi))
            v_loc = it[b_idx, s_idx] - lo
            pairs = np.unique(np.stack([b_idx, v_loc], axis=1), axis=0)
            masked[c][r] = pairs
            max_m = max(max_m, len(pairs))
    w_ext = max(448, ((max_m + 447) // 448) * 448)
    n_scat = max(1, (max_m + BC - 1) // BC)

    E_wT = np.ascontiguousarray(E_w.T)                       # [2H, V]
    lastT = np.ascontiguousarray(last_memory.T)              # [H, B]
    bias = (np.asarray(Ue_b, f32) + np.asarray(We_b, f32)).reshape(H, 1)

    in_maps = []
    for c in range(N_CORES):
        lo = c * VS
        eex0 = np.zeros((H, NCH * w_ext), f32)
        eex1 = np.zeros((H, NCH * w_ext), f32)
        dmask = np.zeros((BC, NCH * w_ext), f32)
        scat = np.full((BC, NCH * n_scat), BC * VS, np.int32)
        for r in range(NCH):
            pairs = masked[c][r]
            m = len(pairs)
            if m:
                vg = pairs[:, 1] + lo
                eex0[:, r * w_ext:r * w_ext + m] = E_wT[:H, vg]
                eex1[:, r * w_ext:r * w_ext + m] = E_wT[H:, vg]
                dmask[pairs[:, 0], r * w_ext + np.arange(m)] = 1.0
                flat = pairs[:, 0] * VS + pairs[:, 1]
                col = np.full(n_scat * BC, BC * VS, np.int64)
                col[:m] = flat
                scat[:, r * n_scat:(r + 1) * n_scat] = col.reshape(
                    n_scat, BC).T
        chunk = slice(c * BC, (c + 1) * BC)
        am_c = all_memory[chunk]                             # [BC, S, H]
        in_maps.append({
            "amT": np.ascontiguousarray(
                am_c.transpose(2, 0, 1).reshape(H, BS)),
            "am_bs": np.ascontiguousarray(am_c.reshape(BC, S * H)),
            "lastT_own": np.ascontiguousarray(lastT[:, chunk]),
            "lastT_bf": lastT.astype(mybir.dt.np(BF16)),
            "UeT": np.ascontiguousarray(np.asarray(Ue_w, f32).T),
            "WeT": np.ascontiguousarray(np.asarray(We_w, f32).T),
            "bias_col": bias,
            "VeCol": np.ascontiguousarray(
                np.asarray(Ve_w, f32).reshape(1, H).T),
            "E0": np.ascontiguousarray(
                E_wT[:H, lo:lo + VS]).astype(mybir.dt.np(BF16)),
            "E1": np.ascontiguousarray(
                E_wT[H:, lo:lo + VS]).astype(mybir.dt.np(BF16)),
            "Eex0": eex0.astype(mybir.dt.np(BF16)),
            "Eex1": eex1.astype(mybir.dt.np(BF16)),
            "Dmask": dmask.astype(mybir.dt.np(BF16)),
            "scat": scat,
        })
    return in_maps, w_ext, n_scat


def kernel(trace=False, **inputs) -> np.ndarray:
    in_maps, w_ext, n_scat = _prep_inputs(**inputs)
    key = (w_ext, n_scat)
    if key not in _cache:
        _cache[key] = _build(w_ext, n_scat)
    nc = _cache[key]
    res = run_bass_kernel_spmd(
        nc, in_maps, core_ids=list(range(N_CORES)), trace=trace
    )
    out = np.empty((B, V), np.float32)
    for c in range(N_CORES):
        for r in range(NCH):
            out[r * BC:(r + 1) * BC, c * VS:(c + 1) * VS] = (
                res.results[c][f"out{r}"]
            )
    kernel.last_exec_time_ns = res.exec_time_ns
    return out
